# revision 7
# baseline (speedup 1.0000x reference)
"""Trainium2 Bass kernel for nn_ContrastiveLoss (N=4096, D=128, NT=512, Q=8).

Strategy (8 NeuronCores, 2 row-groups x 4 col-groups over the [4096 x 8192]
similarity matrix [S_xx | S_xy], memory-regime):
  - Core (r, g) loads only x.T[:, r*2048:(r+1)*2048] (256KB fp8) as the
    lhsT source and cols [g*2048:(g+1)*2048] of [x.T | yf.T] (256KB fp8)
    as the rhs -> 4.3MB total HBM traffic vs 18MB for row-sharding.
  - 16 chunks of [128 rows x 2048 cols]: PE matmul (fp8, optionally
    DoubleRow with a 2x64 k-split for 0.5 cyc/col), then exp row-sums:
    even chunks on ACT (fused exp + per-partition accumulator), odd chunks
    via Schraudolph fast-exp (pass1 tensor_scalar i16 = S*A + B on Pool or
    DVE, pass2 bitcast-bf16 sum on the other engine).
  - Self-pairs killed on device: cores with g == r get dg8 = [-4I | I] and
    a tiny accumulating matmul adds -4 on the diagonal of chunk cc at local
    cols [cc*128, (cc+1)*128); other cores get dg8 = 0 (same SPMD program).
  - Device ships only [128, 16] f32 row sums. The host (f64) computes the
    entire positive path: y[track[i]] dots, sim_p, num, own-track exp sums,
    and the exact same-track x-negative subtraction, then the pair-matrix
    mean via a short convergent series (chunked log1p fallback).
"""

import numpy as np
import ml_dtypes

import concourse.bass as bass
import concourse.bacc as bacc
import concourse.tile as tile
import concourse.mybir as mybir
from concourse import bass_utils

P = 128           # partitions / rows per chunk
N = 4096          # rows of x
D = 128           # feature dim
NT = 512          # number of tracks
Q = 8             # views per track
CORES = 8
RG, CG = 2, 4     # row-groups x col-groups
RROWS = N // RG   # rows per core = 2048
NCH = RROWS // P  # chunks per core = 16
W = 2 * N // CG   # cols per core = 2048
TEMP = 0.05
INV_T = 1.0 / TEMP
DIAG_SHIFT = 4.0  # subtracted on the diagonal pre-exp: exp(-60) == 0
# Schraudolph fast-exp: i16 = S*SCHR_A + SCHR_B, bits-as-bf16 ~= e^(S/T)
SCHR_A = 128.0 * INV_T * 1.4426950408889634
SCHR_B = 16256.5
KAPPA = 1.04069   # mean (1+f)/2^f piecewise-linear bias, divided out on host
# Pool engine can neither read PSUM nor run TensorScalarPtr on this hw, so
# exp row-sums are split ACT vs DVE only: ACT gets 11 tiles (fused exp +
# accumulator), DVE runs Schraudolph pass1+pass2 on 5.
ACT_TILES = frozenset(range(0, NCH, 2)) | {5, 9, 13}
# pass2 variant probe: tensor_reduce may hit the 2x 16-bit DVE mode
REDUCE_P2_TILES = frozenset({1, 3, 7})
DOUBLE_ROW = True
WARMUP_MM = 16

F32 = mybir.dt.float32
BF16 = mybir.dt.bfloat16
I16 = mybir.dt.int16
FP8 = mybir.dt.float8e4
ALU = mybir.AluOpType
ACTF = mybir.ActivationFunctionType

_CACHE = {}


def _build():
    nc = bacc.Bacc("TRN2", target_bir_lowering=False, debug=False,
                   num_devices=CORES)

    if DOUBLE_ROW:
        # [64, 2*2048]: [p, i*2048 + m] = x.T[i*64 + p, row0 + m]
        xh8_d = nc.dram_tensor("xh8", [64, 2 * RROWS], FP8, kind="ExternalInput")
        rhs8_d = nc.dram_tensor("rhs8", [64, 2 * W], FP8, kind="ExternalInput")
    else:
        xh8_d = nc.dram_tensor("xh8", [P, RROWS], FP8, kind="ExternalInput")
        rhs8_d = nc.dram_tensor("rhs8", [P, W], FP8, kind="ExternalInput")
    # [-4*I | I] fp8 on diagonal cores (g == r), zeros elsewhere
    dg8_d = nc.dram_tensor("dg8", [P, 2 * P], FP8, kind="ExternalInput")
    out_d = nc.dram_tensor("out", [P, NCH], F32, kind="ExternalOutput")

    with tile.TileContext(nc) as tc:
        with (
            tc.tile_pool(name="persist", bufs=1) as pp,
            tc.tile_pool(name="escr", bufs=3) as ep,
            tc.tile_pool(name="i16p", bufs=2) as ip,
            tc.tile_pool(name="junkp", bufs=2) as jp,
            tc.tile_pool(name="psum", bufs=2, space="PSUM") as psp,
        ):
            xpart = 64 if DOUBLE_ROW else P
            xh8_s = pp.tile([xpart, (2 * RROWS) if DOUBLE_ROW else RROWS],
                            FP8, tag="xh8_s")
            rhs8_s = pp.tile([xpart, (2 * W) if DOUBLE_ROW else W],
                             FP8, tag="rhs8_s")
            dg8_s = pp.tile([P, 2 * P], FP8, tag="dg8_s")
            outr_s = pp.tile([P, NCH], F32, tag="outr_s")
            ones_s = pp.tile([P, 1], F32, tag="ones_s")
            warm_s = pp.tile([P, 1], F32, tag="warm_s")

            # ---- input loads; DMA-capable queues: sync, scalar, gpsimd ----
            nc.sync.dma_start(out=dg8_s[:], in_=dg8_d.ap())
            wq = rhs8_s.shape[1] // 4
            # rhs quarters in consumption order (DoubleRow: i=0/i=1 halves of
            # the low cols first), split across the scalar + gpsimd queues
            rhs_order = [0, 2, 1, 3] if DOUBLE_ROW else [0, 1, 2, 3]
            for n, qi in enumerate(rhs_order):
                eng = nc.scalar if n % 2 == 0 else nc.gpsimd
                eng.dma_start(out=rhs8_s[:, qi * wq:(qi + 1) * wq],
                              in_=rhs8_d.ap()[:, qi * wq:(qi + 1) * wq])
            xq = xh8_s.shape[1] // 4
            xh_order = [0, 2, 1, 3] if DOUBLE_ROW else [0, 1, 2, 3]
            for qi in xh_order:
                nc.sync.dma_start(out=xh8_s[:, qi * xq:(qi + 1) * xq],
                                  in_=xh8_d.ap()[:, qi * xq:(qi + 1) * xq])

            # pull the exp-table load off the critical path
            nc.vector.memset(ones_s[:], 1.0)
            nc.scalar.activation(out=warm_s[:], in_=ones_s[:], func=ACTF.Exp,
                                 scale=1.0)

            # ---- PE p-state warm-up on dg8 (first DMA to land) ----
            warm_ps = psp.tile([P, W], F32, tag="ps")
            for _ in range(WARMUP_MM):
                nc.tensor.matmul(
                    out=warm_ps[:, 0:2 * P], lhsT=dg8_s[:, 0:P],
                    rhs=dg8_s[:], start=True, stop=True,
                )

            if DOUBLE_ROW:
                xh8_r = xh8_s[:].rearrange("p (i m) -> p i m", i=2)
                rhs8_r = rhs8_s[:].rearrange("p (i n) -> p i n", i=2)

            # ---- main loop: matmul -> exp -> row-sum ----
            for cc in range(NCH):
                ps = psp.tile([P, W], F32, tag="ps")
                for k in range(W // 512):
                    if DOUBLE_ROW:
                        nc.tensor.matmul(
                            out=ps[:, 512 * k:512 * (k + 1)],
                            lhsT=xh8_r[:, :, cc * P:(cc + 1) * P],
                            rhs=rhs8_r[:, :, 512 * k:512 * (k + 1)],
                            start=True, stop=True,
                            perf_mode=mybir.MatmulPerfMode.DoubleRow,
                        )
                    else:
                        nc.tensor.matmul(
                            out=ps[:, 512 * k:512 * (k + 1)],
                            lhsT=xh8_s[:, cc * P:(cc + 1) * P],
                            rhs=rhs8_s[:, 512 * k:512 * (k + 1)],
                            start=True, stop=True,
                        )
                # self-pair kill: S[p, cc*128+p] -= 4 (no-op on dg8==0 cores)
                nc.tensor.matmul(
                    out=ps[:, cc * P:(cc + 1) * P],
                    lhsT=dg8_s[:, 0:P],
                    rhs=dg8_s[:, P:2 * P],
                    start=False, stop=True,
                )
                if cc in ACT_TILES:
                    e = ep.tile([P, W], BF16, tag="escr")
                    nc.scalar.activation(
                        out=e[:], in_=ps[:], func=ACTF.Exp, scale=INV_T,
                        accum_out=outr_s[:, cc:cc + 1],
                    )
                else:
                    i16t = ip.tile([P, W], I16, tag="i16")
                    nc.vector.tensor_scalar(
                        out=i16t[:], in0=ps[:],
                        scalar1=SCHR_A, scalar2=SCHR_B,
                        op0=ALU.mult, op1=ALU.add,
                    )
                    if cc in REDUCE_P2_TILES:
                        nc.vector.tensor_reduce(
                            out=outr_s[:, cc:cc + 1],
                            in_=i16t[:].bitcast(BF16),
                            axis=mybir.AxisListType.X, op=ALU.add,
                        )
                    else:
                        junk = jp.tile([P, W], BF16, tag="junk")
                        nc.vector.tensor_scalar(
                            out=junk[:], in0=i16t[:].bitcast(BF16),
                            scalar1=1.0, scalar2=0.0,
                            op0=ALU.mult, op1=ALU.add,
                            accum_out=outr_s[:, cc:cc + 1],
                        )

            nc.sync.dma_start(out=out_d.ap(), in_=outr_s[:])

    nc.compile()
    return nc


def get_nc():
    if "nc" not in _CACHE:
        _CACHE["nc"] = _build()
    return _CACHE["nc"]


def _dr_pack(a):
    """[128, M] -> [64, 2M] with [p, i*M + m] = a[i*64 + p, m]."""
    m = a.shape[1]
    return np.ascontiguousarray(a.reshape(2, 64, m).transpose(1, 0, 2)
                                .reshape(64, 2 * m))


def prepare_in_maps(x, track_idxs, y):
    x = np.ascontiguousarray(np.asarray(x), dtype=np.float32)
    y = np.ascontiguousarray(np.asarray(y), dtype=np.float32)
    fp8 = ml_dtypes.float8_e4m3
    xT8 = np.ascontiguousarray(x.T.astype(fp8))                  # [128, 4096]
    yT8 = np.ascontiguousarray(y.reshape(N, D).T.astype(fp8))    # [128, 4096]
    colsT8 = np.concatenate([xT8, yT8], axis=1)                  # [128, 8192]
    eye = np.eye(P, dtype=np.float32)
    dg_diag = np.ascontiguousarray(
        np.concatenate([-DIAG_SHIFT * eye, eye], axis=1).astype(fp8))
    dg_zero = np.zeros((P, 2 * P), dtype=fp8)
    in_maps = []
    for c in range(CORES):
        r, g = c // CG, c % CG
        xh8 = np.ascontiguousarray(xT8[:, r * RROWS:(r + 1) * RROWS])
        rhs8 = np.ascontiguousarray(colsT8[:, g * W:(g + 1) * W])
        if DOUBLE_ROW:
            xh8 = _dr_pack(xh8)
            rhs8 = _dr_pack(rhs8)
        in_maps.append({
            "xh8": xh8,
            "rhs8": rhs8,
            "dg8": dg_diag if g == r else dg_zero,
        })
    return in_maps


def combine_outputs(outs, x, track_idxs, y):
    """outs: per-core [128, 16] exp row-sum partials -> scalar loss (host f64)."""
    x = np.asarray(x, np.float64)
    y = np.asarray(y, np.float64)
    t = np.asarray(track_idxs).astype(np.int64)
    kap = np.ones(NCH)
    for cc in range(NCH):
        if cc not in ACT_TILES:
            kap[cc] = KAPPA
    tot = np.zeros(N)
    for c, o in enumerate(outs):
        r = c // CG
        o = np.asarray(o, np.float64) / kap[None, :]             # [128, NCH]
        # row i = r*2048 + cc*128 + p  ->  o[p, cc]
        tot[r * RROWS:(r + 1) * RROWS] += o.T.reshape(-1)
    # host-exact positive path
    dots = np.einsum('id,iqd->iq', x, y[t])                      # [N, Q]
    sim_p = dots.min(axis=1)
    num = np.exp(sim_p * INV_T)
    own = np.exp(dots * INV_T).sum(axis=1)
    # host-exact same-track x negatives
    same_sub = np.zeros(N)
    order = np.argsort(t, kind='stable')
    ts_sorted = t[order]
    starts = np.searchsorted(ts_sorted, np.arange(NT), side='left')
    ends = np.searchsorted(ts_sorted, np.arange(NT), side='right')
    for trk in range(NT):
        idx = order[starts[trk]:ends[trk]]
        if len(idx) < 2:
            continue
        G = x[idx] @ x[idx].T
        E = np.exp(G * INV_T)
        np.fill_diagonal(E, 0.0)
        same_sub[idx] = E.sum(axis=1)
    den = tot - own - same_sub
    if not (np.all(np.isfinite(den)) and np.all(den > 0)):
        raise FloatingPointError("bad den from device")
    # pair term: (1/N^2) sum_ij log(den_j + num_i), via a short series in
    # u_ij = num_i/den_j (< ~0.02 for unit-norm inputs)
    logden = np.log(den)
    pair = N * logden.sum()
    rinv = 1.0 / den
    terms = []
    for k in range(1, 7):
        terms.append((-1.0) ** (k + 1) / k
                     * (num ** k).sum() * (rinv ** k).sum())
    pair += sum(terms)
    if not (abs(terms[-1]) <= 1e-9 * abs(pair) + 1e-12):
        # exact fallback: chunked log1p over the [N, N] ratio matrix
        pair = N * logden.sum()
        for i0 in range(0, N, 512):
            pair += np.log1p(num[i0:i0 + 512, None] * rinv[None, :]).sum()
    loss = pair / (N * N) - sim_p.mean() * INV_T
    return np.float32(loss)


def kernel(x, track_idxs, y):
    nc = get_nc()
    in_maps = prepare_in_maps(x, track_idxs, y)
    res = bass_utils.run_bass_kernel_spmd(nc, in_maps,
                                          core_ids=list(range(CORES)))
    return combine_outputs([r["out"] for r in res.results],
                           x, track_idxs, y)


if __name__ == "__main__":
    nc = get_nc()
    print("build + compile OK")


# revision 14
# speedup vs baseline: 1.1327x; 1.1327x over previous
"""Trainium2 Bass kernel for nn_ContrastiveLoss (N=4096, D=128, NT=512, Q=8).

Strategy (8 NeuronCores, 2 row-groups x 4 col-groups over the [4096 x 8192]
similarity matrix [S_xx | S_xy], memory-regime):
  - Core (r, g) loads only x.T[:, r*2048:(r+1)*2048] (256KB fp8) as the
    lhsT source and cols [g*2048:(g+1)*2048] of [x.T | yf.T] (256KB fp8)
    as the rhs -> 4.3MB total HBM traffic vs 18MB for row-sharding.
  - 16 chunks of [128 rows x 2048 cols]: PE matmul (fp8, optionally
    DoubleRow with a 2x64 k-split for 0.5 cyc/col), then exp row-sums:
    even chunks on ACT (fused exp + per-partition accumulator), odd chunks
    via Schraudolph fast-exp (pass1 tensor_scalar i16 = S*A + B on Pool or
    DVE, pass2 bitcast-bf16 sum on the other engine).
  - Self-pairs killed on device: cores with g == r get dg8 = [-4I | I] and
    a tiny accumulating matmul adds -4 on the diagonal of chunk cc at local
    cols [cc*128, (cc+1)*128); other cores get dg8 = 0 (same SPMD program).
  - Device ships only [128, 16] f32 row sums. The host (f64) computes the
    entire positive path: y[track[i]] dots, sim_p, num, own-track exp sums,
    and the exact same-track x-negative subtraction, then the pair-matrix
    mean via a short convergent series (chunked log1p fallback).
"""

import numpy as np
import ml_dtypes

import concourse.bass as bass
import concourse.bacc as bacc
import concourse.tile as tile
import concourse.mybir as mybir
from concourse import bass_utils

P = 128           # partitions / rows per chunk
N = 4096          # rows of x
D = 128           # feature dim
NT = 512          # number of tracks
Q = 8             # views per track
CORES = 8
RG, CG = 2, 4     # row-groups x col-groups
RROWS = N // RG   # rows per core = 2048
NCH = RROWS // P  # chunks per core = 16
W = 2 * N // CG   # cols per core = 2048
TEMP = 0.05
INV_T = 1.0 / TEMP
DIAG_SHIFT = 4.0  # subtracted on the diagonal pre-exp: exp(-60) == 0
# Schraudolph fast-exp: i16 = S*SCHR_A + SCHR_B, bits-as-bf16 ~= e^(S/T)
SCHR_A = 128.0 * INV_T * 1.4426950408889634
SCHR_B = 16256.5
KAPPA = 1.04069   # mean (1+f)/2^f piecewise-linear bias, divided out on host
# Pool engine can neither read PSUM nor run TensorScalarPtr on this hw, so
# Schraudolph pass1 (PSUM f32 -> i16) always runs on DVE. Pass2 (bf16 free-
# axis sum in SBUF) has per-tile variants, probing for faster paths:
#   act        : ACT fused exp + accumulator (no Schraudolph)
#   schr_acc   : DVE tensor_scalar + accum_out (known 1x)
#   schr_dvep  : DVE pool_avg (InstPool; may hit a faster mode)
#   schr_gpp   : InstPool issued on the GpSimd/Pool engine (ISA probe)
TILE_MODE = {}
for _cc in range(NCH):
    TILE_MODE[_cc] = "act" if _cc % 2 == 0 else "schr_acc"
for _cc in (5, 9, 13):
    TILE_MODE[_cc] = "act"

DOUBLE_ROW = False
WARMUP_MM = 16

F32 = mybir.dt.float32
BF16 = mybir.dt.bfloat16
I16 = mybir.dt.int16
FP8 = mybir.dt.float8e4
ALU = mybir.AluOpType
ACTF = mybir.ActivationFunctionType

_CACHE = {}


def _build():
    nc = bacc.Bacc("TRN2", target_bir_lowering=False, debug=False,
                   num_devices=CORES)

    if DOUBLE_ROW:
        # [64, 2*2048]: [p, i*2048 + m] = x.T[i*64 + p, row0 + m]
        xh8_d = nc.dram_tensor("xh8", [64, 2 * RROWS], FP8, kind="ExternalInput")
        rhs8_d = nc.dram_tensor("rhs8", [64, 2 * W], FP8, kind="ExternalInput")
    else:
        xh8_d = nc.dram_tensor("xh8", [P, RROWS], FP8, kind="ExternalInput")
        rhs8_d = nc.dram_tensor("rhs8", [P, W], FP8, kind="ExternalInput")
    # [-4*I | I] fp8 on diagonal cores (g == r), zeros elsewhere
    dg8_d = nc.dram_tensor("dg8", [P, 2 * P], FP8, kind="ExternalInput")
    out_d = nc.dram_tensor("out", [P, NCH], F32, kind="ExternalOutput")

    with tile.TileContext(nc) as tc:
        with (
            tc.tile_pool(name="persist", bufs=1) as pp,
            tc.tile_pool(name="escr", bufs=3) as ep,
            tc.tile_pool(name="i16p", bufs=2) as ip,
            tc.tile_pool(name="junkp", bufs=2) as jp,
            tc.tile_pool(name="psum", bufs=2, space="PSUM") as psp,
        ):
            xpart = 64 if DOUBLE_ROW else P
            xh8_s = pp.tile([xpart, (2 * RROWS) if DOUBLE_ROW else RROWS],
                            FP8, tag="xh8_s")
            rhs8_s = pp.tile([xpart, (2 * W) if DOUBLE_ROW else W],
                             FP8, tag="rhs8_s")
            dg8_s = pp.tile([P, 2 * P], FP8, tag="dg8_s")
            outr_s = pp.tile([P, NCH], F32, tag="outr_s")
            ones_s = pp.tile([P, 1], F32, tag="ones_s")
            warm_s = pp.tile([P, 1], F32, tag="warm_s")

            # ---- input loads; DMA-capable queues: sync, scalar, gpsimd ----
            nc.sync.dma_start(out=dg8_s[:], in_=dg8_d.ap())
            wh = rhs8_s.shape[1] // 2
            nc.scalar.dma_start(out=rhs8_s[:, 0:wh],
                                in_=rhs8_d.ap()[:, 0:wh])
            nc.gpsimd.dma_start(out=rhs8_s[:, wh:2 * wh],
                                in_=rhs8_d.ap()[:, wh:2 * wh])
            xh = xh8_s.shape[1] // 2
            nc.sync.dma_start(out=xh8_s[:, 0:xh], in_=xh8_d.ap()[:, 0:xh])
            nc.sync.dma_start(out=xh8_s[:, xh:2 * xh],
                              in_=xh8_d.ap()[:, xh:2 * xh])

            # pull the exp-table load off the critical path
            nc.vector.memset(ones_s[:], 1.0)
            nc.scalar.activation(out=warm_s[:], in_=ones_s[:], func=ACTF.Exp,
                                 scale=1.0)

            # ---- PE p-state warm-up on dg8 (first DMA to land) ----
            warm_ps = psp.tile([P, W], F32, tag="ps")
            for _ in range(WARMUP_MM):
                nc.tensor.matmul(
                    out=warm_ps[:, 0:2 * P], lhsT=dg8_s[:, 0:P],
                    rhs=dg8_s[:], start=True, stop=True,
                )

            if DOUBLE_ROW:
                xh8_r = xh8_s[:].rearrange("p (i m) -> p i m", i=2)
                rhs8_r = rhs8_s[:].rearrange("p (i n) -> p i n", i=2)

            # ---- main loop: matmul -> exp -> row-sum ----
            for cc in range(NCH):
                ps = psp.tile([P, W], F32, tag="ps")
                for k in range(W // 512):
                    if DOUBLE_ROW:
                        nc.tensor.matmul(
                            out=ps[:, 512 * k:512 * (k + 1)],
                            lhsT=xh8_r[:, :, cc * P:(cc + 1) * P],
                            rhs=rhs8_r[:, :, 512 * k:512 * (k + 1)],
                            start=True, stop=True,
                            perf_mode=mybir.MatmulPerfMode.DoubleRow,
                        )
                    else:
                        nc.tensor.matmul(
                            out=ps[:, 512 * k:512 * (k + 1)],
                            lhsT=xh8_s[:, cc * P:(cc + 1) * P],
                            rhs=rhs8_s[:, 512 * k:512 * (k + 1)],
                            start=True, stop=True,
                        )
                # self-pair kill: S[p, cc*128+p] -= 4 (no-op on dg8==0 cores)
                nc.tensor.matmul(
                    out=ps[:, cc * P:(cc + 1) * P],
                    lhsT=dg8_s[:, 0:P],
                    rhs=dg8_s[:, P:2 * P],
                    start=False, stop=True,
                )
                mode = TILE_MODE[cc]
                if mode == "act":
                    e = ep.tile([P, W], BF16, tag="escr")
                    nc.scalar.activation(
                        out=e[:], in_=ps[:], func=ACTF.Exp, scale=INV_T,
                        accum_out=outr_s[:, cc:cc + 1],
                    )
                else:
                    i16t = ip.tile([P, W], I16, tag="i16")
                    nc.vector.tensor_scalar(
                        out=i16t[:], in0=ps[:],
                        scalar1=SCHR_A, scalar2=SCHR_B,
                        op0=ALU.mult, op1=ALU.add,
                    )
                    if mode == "schr_dvep":
                        nc.vector.pool(
                            out=outr_s[:, cc:cc + 1],
                            in_=i16t[:].bitcast(BF16).rearrange(
                                "p (a b c n) -> p a b c n", a=1, b=1, c=1),
                            func=mybir.PoolFunctionType.avg,
                        )
                    elif mode == "schr_gpp":
                        bass.BassVectorEngine.pool(
                            nc.gpsimd,
                            out=outr_s[:, cc:cc + 1],
                            in_=i16t[:].bitcast(BF16),
                            func=mybir.PoolFunctionType.avg,
                        )
                    else:
                        junk = jp.tile([P, W], BF16, tag="junk")
                        nc.vector.tensor_scalar(
                            out=junk[:], in0=i16t[:].bitcast(BF16),
                            scalar1=1.0, scalar2=0.0,
                            op0=ALU.mult, op1=ALU.add,
                            accum_out=outr_s[:, cc:cc + 1],
                        )

            nc.sync.dma_start(out=out_d.ap(), in_=outr_s[:])

    nc.compile()
    return nc


def get_nc():
    if "nc" not in _CACHE:
        _CACHE["nc"] = _build()
    return _CACHE["nc"]


def _dr_pack(a):
    """[128, M] -> [64, 2M] with [p, i*M + m] = a[i*64 + p, m]."""
    m = a.shape[1]
    return np.ascontiguousarray(a.reshape(2, 64, m).transpose(1, 0, 2)
                                .reshape(64, 2 * m))


def prepare_in_maps(x, track_idxs, y):
    x = np.ascontiguousarray(np.asarray(x), dtype=np.float32)
    y = np.ascontiguousarray(np.asarray(y), dtype=np.float32)
    fp8 = ml_dtypes.float8_e4m3
    xT8 = np.ascontiguousarray(x.T.astype(fp8))                  # [128, 4096]
    yT8 = np.ascontiguousarray(y.reshape(N, D).T.astype(fp8))    # [128, 4096]
    colsT8 = np.concatenate([xT8, yT8], axis=1)                  # [128, 8192]
    eye = np.eye(P, dtype=np.float32)
    dg_diag = np.ascontiguousarray(
        np.concatenate([-DIAG_SHIFT * eye, eye], axis=1).astype(fp8))
    dg_zero = np.zeros((P, 2 * P), dtype=fp8)
    in_maps = []
    for c in range(CORES):
        r, g = c // CG, c % CG
        xh8 = np.ascontiguousarray(xT8[:, r * RROWS:(r + 1) * RROWS])
        rhs8 = np.ascontiguousarray(colsT8[:, g * W:(g + 1) * W])
        if DOUBLE_ROW:
            xh8 = _dr_pack(xh8)
            rhs8 = _dr_pack(rhs8)
        in_maps.append({
            "xh8": xh8,
            "rhs8": rhs8,
            "dg8": dg_diag if g == r else dg_zero,
        })
    return in_maps


def combine_outputs(outs, x, track_idxs, y):
    """outs: per-core [128, 16] exp row-sum partials -> scalar loss (host f64)."""
    x = np.asarray(x, np.float64)
    y = np.asarray(y, np.float64)
    t = np.asarray(track_idxs).astype(np.int64)
    kap = np.ones(NCH)
    for cc in range(NCH):
        mode = TILE_MODE[cc]
        if mode != "act":
            kap[cc] = KAPPA
        if mode in ("schr_dvep", "schr_gpp"):
            kap[cc] = KAPPA / W          # pool_avg: divide-by-W undone below
    tot = np.zeros(N)
    for c, o in enumerate(outs):
        r = c // CG
        o = np.asarray(o, np.float64) / kap[None, :]             # [128, NCH]
        # row i = r*2048 + cc*128 + p  ->  o[p, cc]
        tot[r * RROWS:(r + 1) * RROWS] += o.T.reshape(-1)
    # host-exact positive path
    dots = np.einsum('id,iqd->iq', x, y[t])                      # [N, Q]
    sim_p = dots.min(axis=1)
    num = np.exp(sim_p * INV_T)
    own = np.exp(dots * INV_T).sum(axis=1)
    # host-exact same-track x negatives
    same_sub = np.zeros(N)
    order = np.argsort(t, kind='stable')
    ts_sorted = t[order]
    starts = np.searchsorted(ts_sorted, np.arange(NT), side='left')
    ends = np.searchsorted(ts_sorted, np.arange(NT), side='right')
    for trk in range(NT):
        idx = order[starts[trk]:ends[trk]]
        if len(idx) < 2:
            continue
        G = x[idx] @ x[idx].T
        E = np.exp(G * INV_T)
        np.fill_diagonal(E, 0.0)
        same_sub[idx] = E.sum(axis=1)
    den = tot - own - same_sub
    if not (np.all(np.isfinite(den)) and np.all(den > 0)):
        raise FloatingPointError("bad den from device")
    # pair term: (1/N^2) sum_ij log(den_j + num_i), via a short series in
    # u_ij = num_i/den_j (< ~0.02 for unit-norm inputs)
    logden = np.log(den)
    pair = N * logden.sum()
    rinv = 1.0 / den
    terms = []
    for k in range(1, 7):
        terms.append((-1.0) ** (k + 1) / k
                     * (num ** k).sum() * (rinv ** k).sum())
    pair += sum(terms)
    if not (abs(terms[-1]) <= 1e-9 * abs(pair) + 1e-12):
        # exact fallback: chunked log1p over the [N, N] ratio matrix
        pair = N * logden.sum()
        for i0 in range(0, N, 512):
            pair += np.log1p(num[i0:i0 + 512, None] * rinv[None, :]).sum()
    loss = pair / (N * N) - sim_p.mean() * INV_T
    return np.float32(loss)


def kernel(x, track_idxs, y):
    nc = get_nc()
    in_maps = prepare_in_maps(x, track_idxs, y)
    res = bass_utils.run_bass_kernel_spmd(nc, in_maps,
                                          core_ids=list(range(CORES)))
    return combine_outputs([r["out"] for r in res.results],
                           x, track_idxs, y)


if __name__ == "__main__":
    nc = get_nc()
    print("build + compile OK")


# revision 19
# speedup vs baseline: 1.1455x; 1.0113x over previous
"""Trainium2 Bass kernel for nn_ContrastiveLoss (N=4096, D=128, NT=512, Q=8).

Strategy (8 NeuronCores, 2 row-groups x 4 col-groups over the [4096 x 8192]
similarity matrix [S_xx | S_xy], memory-regime):
  - Core (r, g) loads only x.T[:, r*2048:(r+1)*2048] (256KB fp8) as the
    lhsT source and cols [g*2048:(g+1)*2048] of [x.T | yf.T] (256KB fp8)
    as the rhs -> 4.3MB total HBM traffic vs 18MB for row-sharding.
  - 16 chunks of [128 rows x 2048 cols]: PE matmul (fp8, optionally
    DoubleRow with a 2x64 k-split for 0.5 cyc/col), then exp row-sums:
    even chunks on ACT (fused exp + per-partition accumulator), odd chunks
    via Schraudolph fast-exp (pass1 tensor_scalar i16 = S*A + B on Pool or
    DVE, pass2 bitcast-bf16 sum on the other engine).
  - Self-pairs killed on device: cores with g == r get dg8 = [-4I | I] and
    a tiny accumulating matmul adds -4 on the diagonal of chunk cc at local
    cols [cc*128, (cc+1)*128); other cores get dg8 = 0 (same SPMD program).
  - Device ships only [128, 16] f32 row sums. The host (f64) computes the
    entire positive path: y[track[i]] dots, sim_p, num, own-track exp sums,
    and the exact same-track x-negative subtraction, then the pair-matrix
    mean via a short convergent series (chunked log1p fallback).
"""

import numpy as np
import ml_dtypes

import concourse.bass as bass
import concourse.bacc as bacc
import concourse.tile as tile
import concourse.mybir as mybir
from concourse import bass_utils

P = 128           # partitions / rows per chunk
N = 4096          # rows of x
D = 128           # feature dim
NT = 512          # number of tracks
Q = 8             # views per track
CORES = 8
RG, CG = 2, 4     # row-groups x col-groups
RROWS = N // RG   # rows per core = 2048
NCH = RROWS // P  # chunks per core = 16
W = 2 * N // CG   # cols per core = 2048
TEMP = 0.05
INV_T = 1.0 / TEMP
DIAG_SHIFT = 4.0  # subtracted on the diagonal pre-exp: exp(-60) == 0
# Schraudolph fast-exp: i16 = S*SCHR_A + SCHR_B, bits-as-bf16 ~= e^(S/T)
SCHR_A = 128.0 * INV_T * 1.4426950408889634
SCHR_B = 16256.5
KAPPA = 1.04069   # mean (1+f)/2^f piecewise-linear bias, divided out on host
# Pool engine can neither read PSUM nor run TensorScalarPtr on this hw, so
# Schraudolph pass1 (PSUM f32 -> i16) always runs on DVE. Pass2 (bf16 free-
# axis sum in SBUF) has per-tile variants, probing for faster paths:
#   act        : ACT fused exp + accumulator (no Schraudolph)
#   schr_acc   : DVE tensor_scalar + accum_out (known 1x)
#   schr_dvep  : DVE pool_avg (InstPool; may hit a faster mode)
#   schr_gpp   : InstPool issued on the GpSimd/Pool engine (ISA probe)
TILE_MODE = {}
for _cc in range(NCH):
    TILE_MODE[_cc] = "act" if _cc % 2 == 0 else "schr_acc"
for _cc in (5, 9, 13):
    TILE_MODE[_cc] = "act"

DOUBLE_ROW = False
WARMUP_MM = 16

F32 = mybir.dt.float32
BF16 = mybir.dt.bfloat16
I16 = mybir.dt.int16
FP8 = mybir.dt.float8e4
ALU = mybir.AluOpType
ACTF = mybir.ActivationFunctionType

_CACHE = {}


def _build():
    nc = bacc.Bacc("TRN2", target_bir_lowering=False, debug=False,
                   num_devices=CORES)

    if DOUBLE_ROW:
        # [64, 2*2048]: [p, i*2048 + m] = x.T[i*64 + p, row0 + m]
        xh8_d = nc.dram_tensor("xh8", [64, 2 * RROWS], FP8, kind="ExternalInput")
        rhs8_d = nc.dram_tensor("rhs8", [64, 2 * W], FP8, kind="ExternalInput")
    else:
        xh8_d = nc.dram_tensor("xh8", [P, RROWS], FP8, kind="ExternalInput")
        rhs8_d = nc.dram_tensor("rhs8", [P, W], FP8, kind="ExternalInput")
    # [-4*I | I] fp8 on diagonal cores (g == r), zeros elsewhere
    dg8_d = nc.dram_tensor("dg8", [P, 2 * P], FP8, kind="ExternalInput")
    # cols [0:NCH] = ACT accumulator sums, [NCH:2*NCH] = DVE pass2 sums.
    # Separate SBUF tiles per engine so Tile's dependency tracking never
    # serializes ACT flushes against DVE flushes (a shared tile chains
    # every consumer and makes the whole main loop serial).
    out_d = nc.dram_tensor("out", [P, 2 * NCH], F32, kind="ExternalOutput")

    with tile.TileContext(nc) as tc:
        with (
            tc.tile_pool(name="persist", bufs=1) as pp,
            tc.tile_pool(name="escr", bufs=3) as ep,
            tc.tile_pool(name="i16p", bufs=2) as ip,
            tc.tile_pool(name="junkp", bufs=2) as jp,
            tc.tile_pool(name="psum", bufs=2, space="PSUM") as psp,
        ):
            xpart = 64 if DOUBLE_ROW else P
            xh8_s = pp.tile([xpart, (2 * RROWS) if DOUBLE_ROW else RROWS],
                            FP8, tag="xh8_s")
            rhs8_s = pp.tile([xpart, (2 * W) if DOUBLE_ROW else W],
                             FP8, tag="rhs8_s")
            dg8_s = pp.tile([P, 2 * P], FP8, tag="dg8_s")
            outa_s = pp.tile([P, NCH], F32, tag="outa_s")
            outv_s = pp.tile([P, NCH], F32, tag="outv_s")
            ones_s = pp.tile([P, 1], F32, tag="ones_s")
            warm_s = pp.tile([P, 1], F32, tag="warm_s")

            # ---- input loads; DMA-capable queues: sync, scalar, gpsimd ----
            nc.sync.dma_start(out=dg8_s[:], in_=dg8_d.ap())
            wh = rhs8_s.shape[1] // 2
            nc.scalar.dma_start(out=rhs8_s[:, 0:wh],
                                in_=rhs8_d.ap()[:, 0:wh])
            nc.gpsimd.dma_start(out=rhs8_s[:, wh:2 * wh],
                                in_=rhs8_d.ap()[:, wh:2 * wh])
            xh = xh8_s.shape[1] // 2
            nc.sync.dma_start(out=xh8_s[:, 0:xh], in_=xh8_d.ap()[:, 0:xh])
            nc.sync.dma_start(out=xh8_s[:, xh:2 * xh],
                              in_=xh8_d.ap()[:, xh:2 * xh])

            # pull the exp-table load off the critical path
            nc.vector.memset(ones_s[:], 1.0)
            nc.scalar.activation(out=warm_s[:], in_=ones_s[:], func=ACTF.Exp,
                                 scale=1.0)

            # ---- PE p-state warm-up on dg8 (first DMA to land) ----
            warm_ps = psp.tile([P, W], F32, tag="ps")
            for _ in range(WARMUP_MM):
                nc.tensor.matmul(
                    out=warm_ps[:, 0:2 * P], lhsT=dg8_s[:, 0:P],
                    rhs=dg8_s[:], start=True, stop=True,
                )

            if DOUBLE_ROW:
                xh8_r = xh8_s[:].rearrange("p (i m) -> p i m", i=2)
                rhs8_r = rhs8_s[:].rearrange("p (i n) -> p i n", i=2)

            # ---- main loop: matmul -> exp -> row-sum ----
            for cc in range(NCH):
                ps = psp.tile([P, W], F32, tag="ps")
                for k in range(W // 512):
                    if DOUBLE_ROW:
                        nc.tensor.matmul(
                            out=ps[:, 512 * k:512 * (k + 1)],
                            lhsT=xh8_r[:, :, cc * P:(cc + 1) * P],
                            rhs=rhs8_r[:, :, 512 * k:512 * (k + 1)],
                            start=True, stop=True,
                            perf_mode=mybir.MatmulPerfMode.DoubleRow,
                        )
                    else:
                        nc.tensor.matmul(
                            out=ps[:, 512 * k:512 * (k + 1)],
                            lhsT=xh8_s[:, cc * P:(cc + 1) * P],
                            rhs=rhs8_s[:, 512 * k:512 * (k + 1)],
                            start=True, stop=True,
                        )
                # self-pair kill: S[p, cc*128+p] -= 4 (no-op on dg8==0 cores)
                nc.tensor.matmul(
                    out=ps[:, cc * P:(cc + 1) * P],
                    lhsT=dg8_s[:, 0:P],
                    rhs=dg8_s[:, P:2 * P],
                    start=False, stop=True,
                )
                mode = TILE_MODE[cc]
                if mode == "act":
                    e = ep.tile([P, W], BF16, tag="escr")
                    nc.scalar.activation(
                        out=e[:], in_=ps[:], func=ACTF.Exp, scale=INV_T,
                        accum_out=outa_s[:, cc:cc + 1],
                    )
                else:
                    i16t = ip.tile([P, W], I16, tag="i16")
                    nc.vector.tensor_scalar(
                        out=i16t[:], in0=ps[:],
                        scalar1=SCHR_A, scalar2=SCHR_B,
                        op0=ALU.mult, op1=ALU.add,
                    )
                    junk = jp.tile([P, W], BF16, tag="junk")
                    nc.vector.tensor_scalar(
                        out=junk[:], in0=i16t[:].bitcast(BF16),
                        scalar1=1.0, scalar2=0.0,
                        op0=ALU.mult, op1=ALU.add,
                        accum_out=outv_s[:, cc:cc + 1],
                    )

            nc.sync.dma_start(out=out_d.ap()[:, 0:NCH], in_=outa_s[:])
            nc.sync.dma_start(out=out_d.ap()[:, NCH:2 * NCH], in_=outv_s[:])

    nc.compile()
    return nc


def get_nc():
    if "nc" not in _CACHE:
        _CACHE["nc"] = _build()
    return _CACHE["nc"]


def _dr_pack(a):
    """[128, M] -> [64, 2M] with [p, i*M + m] = a[i*64 + p, m]."""
    m = a.shape[1]
    return np.ascontiguousarray(a.reshape(2, 64, m).transpose(1, 0, 2)
                                .reshape(64, 2 * m))


def prepare_in_maps(x, track_idxs, y):
    x = np.ascontiguousarray(np.asarray(x), dtype=np.float32)
    y = np.ascontiguousarray(np.asarray(y), dtype=np.float32)
    fp8 = ml_dtypes.float8_e4m3
    xT8 = np.ascontiguousarray(x.T.astype(fp8))                  # [128, 4096]
    yT8 = np.ascontiguousarray(y.reshape(N, D).T.astype(fp8))    # [128, 4096]
    colsT8 = np.concatenate([xT8, yT8], axis=1)                  # [128, 8192]
    eye = np.eye(P, dtype=np.float32)
    dg_diag = np.ascontiguousarray(
        np.concatenate([-DIAG_SHIFT * eye, eye], axis=1).astype(fp8))
    dg_zero = np.zeros((P, 2 * P), dtype=fp8)
    in_maps = []
    for c in range(CORES):
        r, g = c // CG, c % CG
        xh8 = np.ascontiguousarray(xT8[:, r * RROWS:(r + 1) * RROWS])
        rhs8 = np.ascontiguousarray(colsT8[:, g * W:(g + 1) * W])
        if DOUBLE_ROW:
            xh8 = _dr_pack(xh8)
            rhs8 = _dr_pack(rhs8)
        in_maps.append({
            "xh8": xh8,
            "rhs8": rhs8,
            "dg8": dg_diag if g == r else dg_zero,
        })
    return in_maps


def combine_outputs(outs, x, track_idxs, y):
    """outs: per-core [128, 16] exp row-sum partials -> scalar loss (host f64)."""
    x = np.asarray(x, np.float64)
    y = np.asarray(y, np.float64)
    t = np.asarray(track_idxs).astype(np.int64)
    tot = np.zeros(N)
    for c, o in enumerate(outs):
        r = c // CG
        o = np.asarray(o, np.float64)                            # [128, 2*NCH]
        sums = np.empty((P, NCH))
        for cc in range(NCH):
            if TILE_MODE[cc] == "act":
                sums[:, cc] = o[:, cc]
            else:
                sums[:, cc] = o[:, NCH + cc] / KAPPA
        # row i = r*2048 + cc*128 + p  ->  sums[p, cc]
        tot[r * RROWS:(r + 1) * RROWS] += sums.T.reshape(-1)
    # host-exact positive path
    dots = np.einsum('id,iqd->iq', x, y[t])                      # [N, Q]
    sim_p = dots.min(axis=1)
    num = np.exp(sim_p * INV_T)
    own = np.exp(dots * INV_T).sum(axis=1)
    # host-exact same-track x negatives
    same_sub = np.zeros(N)
    order = np.argsort(t, kind='stable')
    ts_sorted = t[order]
    starts = np.searchsorted(ts_sorted, np.arange(NT), side='left')
    ends = np.searchsorted(ts_sorted, np.arange(NT), side='right')
    for trk in range(NT):
        idx = order[starts[trk]:ends[trk]]
        if len(idx) < 2:
            continue
        G = x[idx] @ x[idx].T
        E = np.exp(G * INV_T)
        np.fill_diagonal(E, 0.0)
        same_sub[idx] = E.sum(axis=1)
    den = tot - own - same_sub
    if not (np.all(np.isfinite(den)) and np.all(den > 0)):
        raise FloatingPointError("bad den from device")
    # pair term: (1/N^2) sum_ij log(den_j + num_i), via a short series in
    # u_ij = num_i/den_j (< ~0.02 for unit-norm inputs)
    logden = np.log(den)
    pair = N * logden.sum()
    rinv = 1.0 / den
    terms = []
    for k in range(1, 7):
        terms.append((-1.0) ** (k + 1) / k
                     * (num ** k).sum() * (rinv ** k).sum())
    pair += sum(terms)
    if not (abs(terms[-1]) <= 1e-9 * abs(pair) + 1e-12):
        # exact fallback: chunked log1p over the [N, N] ratio matrix
        pair = N * logden.sum()
        for i0 in range(0, N, 512):
            pair += np.log1p(num[i0:i0 + 512, None] * rinv[None, :]).sum()
    loss = pair / (N * N) - sim_p.mean() * INV_T
    return np.float32(loss)


def kernel(x, track_idxs, y):
    nc = get_nc()
    in_maps = prepare_in_maps(x, track_idxs, y)
    res = bass_utils.run_bass_kernel_spmd(nc, in_maps,
                                          core_ids=list(range(CORES)))
    return combine_outputs([r["out"] for r in res.results],
                           x, track_idxs, y)


if __name__ == "__main__":
    nc = get_nc()
    print("build + compile OK")
